# revision 55
# baseline (speedup 1.0000x reference)
"""BinaryMoSLinear Trainium2 kernel (8-core SPMD, data-parallel over tokens).

Math (per reference):
    xf      = x.reshape(N, H)
    routing = softmax(xf @ gate_w.T)            # [N, E], E = 8
    in_s    = routing @ in_channel_scale        # [N, H]
    out_s   = routing @ out_channel_scale       # [N, O]
    out     = (xf * in_s) @ sign(weight).T * out_s + bias

Device factorization (division-free, all matmuls contract on partitions):
    expT[e, t]   = exp(logitsT[e, t])          (raw, unstabilized; bf16)
    den[t]       = sum_e expT[e, t]            (PE mm with a ones column)
    is_raw[h, t] = sum_e ics[e, h] expT[e, t]  (PE mm, natural ics layout)
    aT[h, t]     = bf16(xT[h, t] * is_raw)     (softmax denom factored out)
    main[t, o]   = sum_h aT[h, t] sign(w)[o, h]
    os_raw[t, o] = sum_e expT[e, t] ocs[e, o]
    out[t, o]    = main * os_raw / den[t]^2 + bias[o]

Each core gets 1024 tokens and the full weight; no collectives.  x and the
binarized weight are transposed on-chip with PE transposes (bf16; sign(w) is
exact in bf16).  The 1/den^2 factor is applied in natural orientation where
t sits on partitions, so it is a per-partition tensor_scalar.
"""

import numpy as np

import concourse.bass as bass
import concourse.mybir as mybir
from concourse import tile
from concourse.bass_utils import run_bass_kernel_spmd
from concourse.masks import make_identity

F32 = mybir.dt.float32
BF16 = mybir.dt.bfloat16
AF = mybir.ActivationFunctionType
ALU = mybir.AluOpType

P = 128
E = 8
N_CORES = 8

# full problem: x [4, 2048, 4096], weight [4096, 4096]
FULL_B, FULL_S, FULL_H, FULL_O = 4, 2048, 4096, 4096
TOK = FULL_B * FULL_S // N_CORES  # 1024 tokens per core

# variant switches (best-measured configuration)
W_XBAR = False   # weight transpose via DMA xbar instead of PE+DVE
X_XBAR = False   # x transpose via DMA xbar instead of PE+DVE
MAIN_N = 512     # moving-operand width of the main matmuls (512 or 1024)


# --------------------------------------------------------------------------
# This container's walrus build accepts at most ONE sync-wait command per
# instruction (DMA descriptors especially).  Tile's scheduler freely stacks
# several waits on one instruction, so rewrite the BIR JSON before compile:
# excess waits become single-wait NoOps immediately preceding the instruction
# on the same engine (program order => identical semantics).
_MAXW = 1


def _split_excess_waits(bir_json: bytes, maxw: int = _MAXW) -> bytes:
    import json as _json

    j = _json.loads(bir_json)
    ctr = 0
    for fn in j["functions"]:
        for blk in fn["blocks"]:
            new = []
            for inst in blk["instructions"]:
                si = inst.get("sync_info")
                if si:
                    waits = si.get("on_wait") or []
                    if len(waits) > maxw:
                        extra, keep = waits[:-maxw], waits[-maxw:]
                        for i in range(0, len(extra), maxw):
                            ctr += 1
                            nop = {
                                "name": f"I-wsplit-{ctr}",
                                "opcode": "NoOp",
                                "engine": inst["engine"],
                                "ins": [],
                                "outs": [],
                                "sync_info": {
                                    "on_wait": extra[i : i + maxw],
                                    "on_update": [],
                                },
                            }
                            if "debug" in inst:
                                nop["debug"] = inst["debug"]
                            new.append(nop)
                        si["on_wait"] = keep
                new.append(inst)
            blk["instructions"] = new
    return _json.dumps(j).encode()


def _install_wait_split():
    from concourse import bass2jax, bass_utils

    orig = bass_utils.compile_bir_kernel
    if getattr(orig, "_wait_split_wrapped", False):
        return

    def wrapped(bir_json, tmpdir, neff_name="file.neff"):
        return orig(_split_excess_waits(bir_json), tmpdir, neff_name)

    wrapped._wait_split_wrapped = True
    bass_utils.compile_bir_kernel = wrapped
    bass2jax.compile_bir_kernel = wrapped


_install_wait_split()
# --------------------------------------------------------------------------


def build_nc(tok=TOK, h=FULL_H, o=FULL_O, w_xbar=None, x_xbar=None, main_n=None,
             probe=()):
    """Build the per-core Bass program.  tok/h/o shrinkable for debugging.

    probe: timing-probe switches (correctness-breaking, bench only):
      'no_wdma'  - skip weight load DMAs
      'no_xdma'  - skip x load DMAs (beyond the first prefetched strip)
      'no_store' - skip output stores except the first block
    """
    w_xbar = W_XBAR if w_xbar is None else w_xbar
    x_xbar = X_XBAR if x_xbar is None else x_xbar
    main_n = MAIN_N if main_n is None else main_n
    probe = set(probe)
    HC = h // P          # 128-wide h-chunks
    TB = tok // P        # 128-token blocks
    TH = tok // 512      # 512-token halves (in_scale granularity)
    ON = main_n          # main-mm moving width
    OC = o // ON         # output chunks
    NS = ON // P         # o-strips per stage
    JH = 8192 // ON      # h-chunks per wbT stage (stage = 16KB/partition)
    HH = HC // JH        # wbT stages per output chunk
    TG = (4 * 512) // ON  # concurrent psum accumulations (4 banks total)
    assert tok % 512 == 0 and h % (JH * P) == 0 and o % ON == 0
    assert TB % TG == 0

    nc = bass.Bass("TRN2", target_bir_lowering=False, debug=False,
                   num_devices=N_CORES)

    HC_ = h // P
    x_d = nc.declare_dram_parameter("x", [tok, h], F32, isOutput=False)
    w_d = nc.declare_dram_parameter("weight", [o, h], F32, isOutput=False)
    b_d = nc.declare_dram_parameter("bias", [o], F32, isOutput=False)
    # gate_w pre-transposed host-side to [128, HC*E]: column block hc holds
    # gate_w[:, hc*128:(hc+1)*128].T  (pure layout transform, done once on
    # the host in core_in_maps)
    gwt_d = nc.declare_dram_parameter("gwt", [P, HC_ * E], F32, isOutput=False)
    ics_d = nc.declare_dram_parameter("ics", [E, h], F32, isOutput=False)
    ocs_d = nc.declare_dram_parameter("ocs", [E, o], F32, isOutput=False)
    out_d = nc.declare_dram_parameter("out", [tok, o], F32, isOutput=True)

    with tile.TileContext(nc) as tc:
        with (
            tc.tile_pool(name="const", bufs=1) as const,
            tc.tile_pool(name="sb", bufs=2) as sb,
            tc.tile_pool(name="wsgn", bufs=NS) as wsgnp,
            tc.tile_pool(name="wbt", bufs=3) as wbtp,
            tc.tile_pool(name="ossb", bufs=1) as ossbp,
            tc.tile_pool(name="pmm", bufs=TG, space="PSUM") as pmm,
            tc.tile_pool(name="pos", bufs=1, space="PSUM") as posp,
            tc.tile_pool(name="pT", bufs=3, space="PSUM") as pT,
        ):
            psmall = posp  # share the single small-PSUM bank (phase-disjoint)
            # ---- constants / persistent tiles ----
            # request the first x strips before any const prep so the Pool
            # DMA queue feeds PE as early as possible
            x_bf_tiles = {}
            # first strip via HWDGE (fast start) + DVE cast; SWDGE cast-DMA
            # has a ~6us first-byte cost that would idle PE at kernel start
            def load_strip_hwdge(x_bf, t0):
                # quarters alternating over both HWDGE queues + DVE cast
                for ci in range(4):
                    c0 = ci * (h // 4)
                    xf32 = sb.tile([P, h // 4], F32, tag="xf32",
                                   bufs=2, name=f"xf32_{t0}_{ci}")
                    eng = nc.sync if ci % 2 == 0 else nc.scalar
                    eng.dma_start(
                        out=xf32, in_=x_d[t0 : t0 + P, c0 : c0 + h // 4]
                    )
                    nc.vector.tensor_copy(
                        out=x_bf[:, c0 : c0 + h // 4], in_=xf32
                    )

            x_bf_tiles[0] = sb.tile([P, h], BF16, tag="xbf", bufs=2,
                                    name="xbf_pre0")
            load_strip_hwdge(x_bf_tiles[0], 0)
            if TB > 1:
                x_bf_tiles[1] = sb.tile([P, h], BF16, tag="xbf", bufs=2,
                                        name="xbf_pre1")
                for ci in range(4):
                    c0 = ci * (h // 4)
                    nc.gpsimd.dma_start(
                        out=x_bf_tiles[1][:, c0 : c0 + h // 4],
                        in_=x_d[P : 2 * P, c0 : c0 + h // 4],
                    )

            id_bf = const.tile([P, P], BF16, name="id_bf")
            make_identity(nc, id_bf)
            ones_bf = const.tile([P, 1], BF16, name="ones_bf")
            nc.vector.memset(ones_bf, 1.0)

            aT = const.tile([P, HC * tok], BF16, name="aT")
            aT3 = aT.rearrange("p (hc t) -> p hc t", t=tok)
            # E-contraction matmuls below slice [0:E] partitions, so rows
            # E:128 of expT/ics/ocs are never read -> no zero-fill memsets
            expT = const.tile([P, tok], BF16, name="expT")
            invden = const.tile([P, TB], F32, name="invden")
            invden2 = const.tile([P, TB], F32, name="invden2")

            gwT = const.tile([P, HC * E], BF16, name="gwT")
            ics_bf = const.tile([P, h], BF16, name="ics_bf")
            ocs_bf = const.tile([P, o], BF16, name="ocs_bf")
            bias_bc = const.tile([P, o], BF16, name="bias_bc")

            # ---- weight stage load (defined early; stage 0 issued before
            # phase A so SP/ACT HWDGE queues + ACT Signs warm up under the
            # x transposes) ----
            def stage_load(oc, hh):
                o0 = oc * ON
                wsgn = []
                for st in range(NS):
                    wf = sb.tile([P, JH * P], F32, tag="wf32",
                                 name=f"wf_{oc}_{hh}_{st}")
                    if "no_wdma" not in probe:
                        # alternate the two HWDGE queues (SP / ACT)
                        eng = nc.sync if (st % 2 == 0) else nc.scalar
                        eng.dma_start(
                            out=wf,
                            in_=w_d[
                                o0 + st * P : o0 + (st + 1) * P,
                                hh * JH * P : (hh + 1) * JH * P,
                            ],
                        )
                    ws = wsgnp.tile([P, JH * P], BF16, tag="wsgn",
                                    name=f"ws_{oc}_{hh}_{st}")
                    nc.scalar.activation(ws, wf, AF.Sign)
                    wsgn.append(ws)
                return wsgn

            wsgn_next = [stage_load(0, 0)]

            def stage_transpose_ops(oc, hh, wsgn):
                """Yield thunks, each emitting one transpose-group + copy."""
                wt = wbtp.tile([P, JH * ON], BF16, tag="wbt",
                               name=f"wt_{oc}_{hh}")

                def make(j, g):
                    def emit():
                        if w_xbar:
                            for k in range(4):
                                st = g * 4 + k
                                nc.scalar.dma_start(
                                    out=wt[:, j * ON + st * P : j * ON + (st + 1) * P],
                                    in_=wsgn[st][:, j * P : (j + 1) * P],
                                    transpose=True,
                                )
                        else:
                            pt = pT.tile([P, 512], BF16, tag="T4",
                                         name=f"ptw_{oc}_{hh}_{j}_{g}")
                            for k in range(4):
                                nc.tensor.transpose(
                                    pt[:, k * P : (k + 1) * P],
                                    wsgn[g * 4 + k][:, j * P : (j + 1) * P],
                                    id_bf,
                                )
                            # evacuate on ACT (idle during mains) so the pT
                            # ring never waits on DVE's epilogue bursts
                            nc.scalar.activation(
                                wt[:, j * ON + g * 512 : j * ON + (g + 1) * 512],
                                pt,
                                AF.Copy,
                            )
                    return emit

                thunks = [make(j, g) for j in range(JH) for g in range(NS // 4)]
                return wt, thunks

            # only stage-0/hh0 must be transposed before the mains start;
            # hh>0 thunks run urgently inside oc0's first mains, so phase A
            # competes with 8MB of weight DMA instead of 16MB
            s0_wbt, s0_thunks, s0_rest = [], [], []
            wt, thunks = stage_transpose_ops(0, 0, wsgn_next[0])
            s0_wbt.append(wt)
            s0_thunks.extend(thunks)
            s0_i = 0


            # ---- phase A: x -> xT -> gating -> aT (scaled, bf16) ----
            # sub-phase 1: transpose all x blocks (dense PE work)
            for tb in range(TB):
                t0 = tb * P
                if tb in x_bf_tiles:
                    x_bf = x_bf_tiles.pop(tb)
                else:
                    x_bf = sb.tile([P, h], BF16, tag="xbf", bufs=2)
                    if "no_xdma" not in probe:
                        for ci in range(4):
                            c0 = ci * (h // 4)
                            nc.gpsimd.dma_start(
                                out=x_bf[:, c0 : c0 + h // 4],
                                in_=x_d[t0 : t0 + P, c0 : c0 + h // 4],
                            )
                if x_xbar:
                    for hc in range(HC):
                        nc.scalar.dma_start(
                            out=aT[:, hc * tok + t0 : hc * tok + t0 + P],
                            in_=x_bf[:, hc * P : (hc + 1) * P],
                            transpose=True,
                        )
                else:
                    for j0 in range(0, HC, 4):
                        pt = pT.tile([P, 512], BF16, tag="T4")
                        for k in range(4):
                            nc.tensor.transpose(
                                pt[:, k * P : (k + 1) * P],
                                x_bf[:, (j0 + k) * P : (j0 + k + 1) * P],
                                id_bf,
                            )
                        nc.vector.tensor_copy(
                            out=aT3[:, j0 : j0 + 4, t0 : t0 + P],
                            in_=pt.rearrange("p (b t) -> p b t", t=P),
                        )
                if tb >= 1:
                    # fill x-supply gaps with stage-0/hh0 weight transposes
                    take = min(3, len(s0_thunks) - s0_i)
                    for _ in range(take):
                        s0_thunks[s0_i]()
                        s0_i += 1
            # const loads on the Pool SWDGE queue, after the x strips so they
            # don't delay the transposes' data supply; all are only needed
            # from sub-phase 2 (gating) onward
            nc.gpsimd.dma_start(out=gwT, in_=gwt_d[:, :])
            nc.gpsimd.dma_start(out=ics_bf[0:E, :], in_=ics_d[:, :])
            nc.gpsimd.dma_start(out=ocs_bf[0:E, :], in_=ocs_d[:, :])
            nc.gpsimd.dma_start(
                out=bias_bc, in_=b_d[None, :].to_broadcast((P, o))
            )
            # stage-0 upper halves: loads deferred until all x strips are
            # queued so phase A's x supply wins the DMA pool; transposes run
            # urgently inside oc0's first mains
            for hh in range(1, HH):
                wsgn_next.append(stage_load(0, hh))
                wt, thunks = stage_transpose_ops(0, hh, wsgn_next[hh])
                s0_wbt.append(wt)
                s0_rest.extend(thunks)

            # sub-phase 2: gating (logits, exp, denominators); logits done
            # 512 tokens at a time (fewer PE instructions, same cycles)
            for th in range(TH):
                s0 = th * 512
                pl = psmall.tile([E, 512], F32, tag="os", name="pl")
                for hc in range(HC):
                    nc.tensor.matmul(
                        pl,
                        gwT[:, hc * E : (hc + 1) * E],
                        aT[:, hc * tok + s0 : hc * tok + s0 + 512],
                        start=(hc == 0),
                        stop=(hc == HC - 1),
                    )
                nc.scalar.activation(expT[0:E, s0 : s0 + 512], pl, AF.Exp)
                for tb in range(th * 4, th * 4 + 4):
                    t0 = tb * P
                    pd = psmall.tile([P, 1], F32, tag="os", name="pd")
                    nc.tensor.matmul(
                        pd, expT[0:E, t0 : t0 + P], ones_bf[0:E],
                        start=True, stop=True,
                    )
                    nc.vector.reciprocal(invden[:, tb : tb + 1], pd)
                    nc.vector.tensor_tensor(
                        invden2[:, tb : tb + 1],
                        invden[:, tb : tb + 1],
                        invden[:, tb : tb + 1],
                        ALU.mult,
                    )
            # sub-phase 3: fold is_raw into aT
            for th in range(TH):
                s0 = th * 512
                for hc in range(HC):
                    pis = pmm.tile([P, 512], F32, tag="mm")
                    nc.tensor.matmul(
                        pis,
                        ics_bf[0:E, hc * P : (hc + 1) * P],
                        expT[0:E, s0 : s0 + 512],
                        start=True,
                        stop=True,
                    )
                    sl = aT[:, hc * tok + s0 : hc * tok + s0 + 512]
                    nc.vector.tensor_tensor(sl, sl, pis, ALU.mult)

            # ---- phase C: weight sign+transpose + main matmul + epilogue ----
            # Stage (oc, hh) = JH h-chunks x ON o-cols of the binarized,
            # transposed weight.  Loads+signs are issued an oc ahead; the PE
            # transposes for chunk oc+1 are interleaved into chunk oc's main
            # matmul stream so PE never sits waiting on the DVE evacuation
            # copies at a chunk boundary.
            # prologue: stage 0 was loaded + transposed during phase A;
            # flush any leftover transpose thunks
            while s0_i < len(s0_thunks):
                s0_thunks[s0_i]()
                s0_i += 1
            wbt = s0_wbt

            for oc in range(OC):
                o0 = oc * ON
                # next chunk: issue loads+signs now, interleave transposes
                # into this chunk's mains (one group per `stride` mains)
                # precompute this chunk's out_scale rows into SBUF (bf16),
                # interleaved into the mains stream: the epilogue then reads
                # SBUF instead of serializing on the single pos PSUM slot
                os_sb = ossbp.tile([P, TB * ON], BF16, tag="ossb",
                                   name=f"ossb_{oc}")

                def make_os(tb, q0, dst):
                    def emit():
                        pos = posp.tile([P, 512], F32, tag="os",
                                        name=f"pos_{oc}_{tb}_{q0}")
                        nc.tensor.matmul(
                            pos,
                            expT[0:E, tb * P : (tb + 1) * P],
                            ocs_bf[0:E, q0 : q0 + 512],
                            start=True,
                            stop=True,
                        )
                        nc.vector.tensor_copy(out=dst, in_=pos)
                    return emit

                pending = []
                urgent = list(s0_rest) if oc == 0 else []
                s0_rest = []
                for tb in range(TB):
                    for half in range(ON // 512):
                        pending.append(make_os(
                            tb,
                            o0 + half * 512,
                            os_sb[:, tb * ON + half * 512 : tb * ON + (half + 1) * 512],
                        ))
                if oc + 1 < OC:
                    nxt = [stage_load(oc + 1, hh) for hh in range(HH)]
                    next_wbt = []
                    for hh in range(HH):
                        wt, thunks = stage_transpose_ops(oc + 1, hh, nxt[hh])
                        next_wbt.append(wt)
                        pending.extend(thunks)
                n_mains = (TB // TG) * HH * TG * JH
                stride = max(1, n_mains // max(1, len(pending))) if pending else 0
                mi = 0
                for tg in range(TB // TG):
                    tbs = list(range(tg * TG, (tg + 1) * TG))
                    pms = [pmm.tile([P, ON], F32, tag="mm",
                                    name=f"pm_{oc}_{tg}_{i}")
                           for i in range(len(tbs))]
                    for hh in range(HH):
                        for i, tb in enumerate(tbs):
                            t0 = tb * P
                            for j in range(JH):
                                hc = hh * JH + j
                                nc.tensor.matmul(
                                    pms[i],
                                    aT[:, hc * tok + t0 : hc * tok + t0 + P],
                                    wbt[hh][:, j * ON : (j + 1) * ON],
                                    start=(hh == 0 and j == 0),
                                    stop=(hh == HH - 1 and j == JH - 1),
                                )
                                mi += 1
                                if urgent:
                                    urgent.pop(0)()
                                elif pending and stride and mi % stride == 0:
                                    pending.pop(0)()
                    for i, tb in enumerate(tbs):
                        t0 = tb * P
                        for half in range(ON // 512):
                            q0 = o0 + half * 512
                            pos = os_sb[:, tb * ON + half * 512 : tb * ON + (half + 1) * 512]
                            tmp = sb.tile([P, 512], F32, tag="out", bufs=2)
                            # evacuate psum_main on DVE, folding in 1/den^2 as
                            # a per-partition scalar; keeps ACT free for Sign
                            nc.vector.tensor_scalar_mul(
                                tmp,
                                pms[i][:, half * 512 : (half + 1) * 512],
                                invden2[:, tb : tb + 1],
                            )
                            nc.vector.tensor_tensor(tmp, tmp, pos, ALU.mult)
                            # bias add on the near-idle GPSIMD engine (SBUF
                            # operands only), shortening the DVE chain that
                            # holds the pos PSUM slot
                            nc.gpsimd.tensor_tensor(
                                tmp, tmp, bias_bc[:, q0 : q0 + 512], ALU.add
                            )
                            # store on the (otherwise idle) SWDGE/Pool queue
                            # so a blocked store never stalls the next
                            # chunk's weight loads in the SP HWDGE FIFO.
                            # Last chunk: no more weight loads coming, spread
                            # stores over all three queues to shorten the tail.
                            if "no_store" not in probe and not (
                                "no_store_most" in probe and (tb or oc)
                            ):
                                if oc == OC - 1:
                                    seng = (nc.gpsimd, nc.sync, nc.scalar)[
                                        tb % 3
                                    ]
                                else:
                                    seng = nc.gpsimd
                                seng.dma_start(
                                    out=out_d[t0 : t0 + P, q0 : q0 + 512],
                                    in_=tmp,
                                )
                for th_ in urgent + pending:
                    th_()
                if oc + 1 < OC:
                    wbt = next_wbt
    return nc


_NC_CACHE = {}


def _get_nc(key=None):
    if key is None:
        key = (TOK, FULL_H, FULL_O, W_XBAR, X_XBAR, MAIN_N)
    if key not in _NC_CACHE:
        _NC_CACHE[key] = build_nc(*key)
    return _NC_CACHE[key]


def make_in_maps(x, weight, bias, gate_w, in_channel_scale, out_channel_scale):
    """Host-side prep: shard x over cores, pre-transpose gate_w (layout only)."""
    H = x.shape[-1]
    xf = np.ascontiguousarray(x.reshape(-1, H).astype(np.float32, copy=False))
    weight = np.ascontiguousarray(weight.astype(np.float32, copy=False))
    bias = np.ascontiguousarray(bias.astype(np.float32, copy=False))
    gate_w = np.asarray(gate_w, dtype=np.float32)
    # gwt[p, hc*E + e] = gate_w[e, hc*128 + p]
    gwt = np.ascontiguousarray(
        gate_w.reshape(E, H // P, P).transpose(2, 1, 0).reshape(P, -1)
    )
    ics = np.ascontiguousarray(in_channel_scale.astype(np.float32, copy=False))
    ocs = np.ascontiguousarray(out_channel_scale.astype(np.float32, copy=False))
    return [
        {
            "x": xf[c * TOK : (c + 1) * TOK],
            "weight": weight,
            "bias": bias,
            "gwt": gwt,
            "ics": ics,
            "ocs": ocs,
        }
        for c in range(N_CORES)
    ]


def kernel(x, weight, bias, gate_w, in_channel_scale, out_channel_scale):
    B, S, H = x.shape
    in_maps = make_in_maps(
        x, weight, bias, gate_w, in_channel_scale, out_channel_scale
    )
    nc = _get_nc()
    res = run_bass_kernel_spmd(nc, in_maps, list(range(N_CORES)))
    out = np.concatenate(
        [res.results[c]["out"] for c in range(N_CORES)], axis=0
    )
    return out.reshape(B, S, -1)

